# revision 26
# baseline (speedup 1.0000x reference)
"""Trainium2 Bass kernel for nn_GRULayer (Keras GRU, reset_after=True, Masking(0)).

Data-parallel over batch (8 cores, B_loc=32). Per core the 512-step scan is
latency-optimized; per step the critical chain is:

  v16 -> rec matmuls (v-part) -> sigmoid(r) -> t16 = r*mhh (DVE) ->
  ident-matmul accumulate (hcp = xh + t16, PE) -> sigmoid(s) ->
  tensor_tensor_scan (v = q*(2s-u), DVE) -> next step

Key tricks:
  - state u = h + 1 (f32 master u32; bf16 u at slot0 of the scan's b-tile);
    v(g) = u(g) - u(g-1). Recurrence matmul split: Uk^T u(g) =
    Uk^T u(g-1) [issued early, off path] + Uk^T v(g) [on path].
  - z-gate negated host-side: sigmoid gives q = 1-z; mask folds in as
    -30*(1-mask) so masked steps give q=0 (h carried).
  - candidate tanh via sigmoid: weights pre-scaled x2, hc = 2s-1; update
    u' = u + q*(2s - u) computed in ONE tensor_tensor_scan over
    [slot0, slot1, slot2] triples with a=[0,-0.5,2q], b=[u,s,0].
  - PSUM discipline: exactly one start=True per psum tile refill (the
    start arms the whole 2KB zero region; later matmuls init-then-
    accumulate per address).
  - projection, biases (ones-matmuls), mask, and next-step u-part matmuls
    all run in PE idle windows; projection is drip-fed between steps.

Host prep: z-block weights negated, candidate block scaled by 2, u-shift
constants (colsum of Uk) folded into biases. Output: h = u32 - 1,
PE-transposed to [batch, units].
"""

import os
import sys

sys.path.insert(0, "/opt/trn_rl_repo")

import ml_dtypes
import numpy as np

import concourse.bass as bass
import concourse.mybir as mybir
import concourse.tile as tile
from concourse import bacc
from concourse.bass_utils import run_bass_kernel_spmd
from concourse.masks import make_identity

B, T, D, U = 256, 512, 256, 256
NCORES = 8
BLOC = B // NCORES  # 32
KT = D // 128
KU = U // 128
S = 4  # steps per psum chunk
NCHUNKS = T // S  # 128
CHUNK_COLS = S * BLOC  # 128
DMA_STEPS = 32  # steps per DMA batch
DMA_COLS = DMA_STEPS * BLOC  # 1024
CHUNKS_PER_DMA = DMA_STEPS // S  # 8
N_DMA = T // DMA_STEPS  # 16
MASK_BIG = 30.0

F32 = mybir.dt.float32
BF16 = mybir.dt.bfloat16
SIG = mybir.ActivationFunctionType.Sigmoid

# column blocks in w/uk/bias (m-tile index):
#   0:2 -> candidate (2*Wh / 2*Ukh) -> P2[:,0:2]
#   2:4 -> zn (-Wz / -Ukz)          -> P2[:,2:4]
#   4:6 -> r  (Wr / Ukr)            -> P1
M_XH, M_ZN, M_R = 0, 2, 4

LAST_RESULTS = None  # test.py reads exec_time_ns off this
DEBUG_DUMP = False  # dump per-step gate intermediates to DRAM


def _build_program():
    nc = bacc.Bacc(
        "TRN2", target_bir_lowering=False, debug=False, num_devices=NCORES
    )

    codesT_d = nc.dram_tensor("codesT", [KT, 128, T * BLOC], BF16, kind="ExternalInput")
    minf_d = nc.dram_tensor("minf", [1, T * BLOC], BF16, kind="ExternalInput")
    w_d = nc.dram_tensor("w", [KT, 128, 768], BF16, kind="ExternalInput")
    uk_d = nc.dram_tensor("uk", [KU, 128, 768], BF16, kind="ExternalInput")
    bias_d = nc.dram_tensor("bias", [1, 768], BF16, kind="ExternalInput")
    hconst_d = nc.dram_tensor("hconst", [1, 256], BF16, kind="ExternalInput")
    out_d = nc.dram_tensor("out", [128, KU, BLOC], F32, kind="ExternalOutput")
    dbg = {}
    if DEBUG_DUMP:
        for nm, d1 in (("rq", 4), ("t16", KU), ("hcp16", KU), ("s16", KU),
                       ("w16", KU), ("v16", KU), ("u16d", KU)):
            dbg[nm] = nc.dram_tensor(f"dbg_{nm}", [T, 128, d1, BLOC], BF16,
                                     kind="ExternalOutput")
        for nm, d1 in (("p1", 4), ("p2", KU), ("p3", KU)):
            dbg[nm] = nc.dram_tensor(f"dbg_{nm}", [T, 128, d1, BLOC], F32,
                                     kind="ExternalOutput")

    with tile.TileContext(nc) as tc:
        with (
            tc.tile_pool(name="const", bufs=1) as const,
            tc.tile_pool(name="cin", bufs=3) as cin,
            tc.tile_pool(name="gate", bufs=3) as gate,
            tc.tile_pool(name="state", bufs=1) as state,
            tc.tile_pool(name="p1", bufs=2, space="PSUM") as p1p,
            tc.tile_pool(name="p2", bufs=2, space="PSUM") as p2p,
            tc.tile_pool(name="p3", bufs=2, space="PSUM") as p3p,
        ):
            # ---- constants ----
            w_sb = [const.tile([128, 768], BF16, tag=f"w{k}", name=f"w{k}") for k in range(KT)]
            uk_sb = [const.tile([128, 768], BF16, tag=f"uk{k}", name=f"uk{k}") for k in range(KU)]
            for k in range(KT):
                nc.sync.dma_start(out=w_sb[k], in_=w_d[k])
            for k in range(KU):
                nc.sync.dma_start(out=uk_sb[k], in_=uk_d[k])
            bias_sb = const.tile([1, 768], BF16, tag="bias")
            nc.sync.dma_start(out=bias_sb, in_=bias_d[0:1, :])
            hconst_sb = const.tile([1, 256], BF16, tag="hconst")
            nc.sync.dma_start(out=hconst_sb, in_=hconst_d[0:1, :])
            ones_sb = const.tile([1, CHUNK_COLS], BF16, tag="ones")
            nc.vector.memset(ones_sb, 1.0)
            identb = const.tile([128, 128], BF16, tag="identb")
            make_identity(nc, identb)

            # ---- state ----
            # tts layout: per element-triple [slot0, slot1, slot2]:
            #   a = [0, -0.5, 2q],  b = [u, s, 0]
            #   state: u -> s - u/2 = w/2 -> 2q*w/2 = q*(2s-u) = v
            u32 = state.tile([128, KU, BLOC], F32, tag="u32")
            nc.vector.memset(u32, 1.0)
            atile = state.tile([128, KU * BLOC, 3], BF16, tag="atile")
            nc.vector.memset(atile, 0.0)
            nc.vector.memset(atile[:, :, 1], -0.5)
            btile = state.tile([128, KU * BLOC, 3], BF16, tag="btile")
            nc.vector.memset(btile, 0.0)
            nc.vector.memset(btile[:, :, 0], 1.0)  # u0 = 1
            vtile = state.tile([128, KU * BLOC, 3], BF16, tag="vtile")
            # dense-view helpers: u lives strided at slot 0
            u16v = btile[:, :, 0].rearrange("p (k b) -> p k b", b=BLOC)

            # ---- DMA prefetch machinery ----
            ct_bufs = {}

            def issue_dma(bidx):
                if bidx >= N_DMA or bidx in ct_bufs:
                    return
                d0 = bidx * DMA_COLS
                cts = [
                    cin.tile([128, DMA_COLS], BF16, tag=f"ct{k}", name=f"ct{k}")
                    for k in range(KT)
                ]
                for k in range(KT):
                    nc.sync.dma_start(out=cts[k], in_=codesT_d[k, :, d0 : d0 + DMA_COLS])
                mi = cin.tile([1, DMA_COLS], BF16, tag="mi")
                nc.sync.dma_start(out=mi, in_=minf_d[0:1, d0 : d0 + DMA_COLS])
                ct_bufs[bidx] = (cts, mi)

            issue_dma(0)
            issue_dma(1)

            def proj(c):
                """Allocate psum tiles for chunk c and write projection."""
                bidx = (c * S) // DMA_STEPS
                off = (c % CHUNKS_PER_DMA) * CHUNK_COLS
                cts, mi = ct_bufs[bidx]
                P1 = p1p.tile([128, 4, S, BLOC], F32, tag="P1")
                P2 = p2p.tile([128, KU, S, BLOC], F32, tag="P2")

                def pslice(j):
                    if j < 4:
                        return P1[:, j].rearrange("p s b -> p (s b)")
                    return P2[:, j - 4].rearrange("p s b -> p (s b)")

                # P1 j0:2 = r (m 4:6), P1 j2:4 = zn (m 2:4), P2 = xh (m 0:2)
                thunks = []
                for j, m in ((0, M_R), (1, M_R + 1), (2, M_ZN), (3, M_ZN + 1),
                             (4, M_XH), (5, M_XH + 1)):
                    dst = pslice(j)
                    for k in range(KT):
                        # exactly ONE start=True per psum tile refill: the
                        # first matmul arms the whole zero region; later
                        # writes init-then-accumulate per address
                        thunks.append(
                            lambda dst=dst, m=m, k=k, j=j: nc.tensor.matmul(
                                dst,
                                w_sb[k][:, m * 128 : (m + 1) * 128],
                                cts[k][:, off : off + CHUNK_COLS],
                                start=(j in (0, 4) and k == 0),
                                stop=False,
                                skip_group_check=True,
                            )
                        )
                    thunks.append(
                        lambda dst=dst, m=m: nc.tensor.matmul(
                            dst,
                            bias_sb[:, m * 128 : (m + 1) * 128],
                            ones_sb,
                            start=False,
                            stop=False,
                            skip_group_check=True,
                        )
                    )
                    if m in (M_ZN, M_ZN + 1):
                        thunks.append(
                            lambda dst=dst: nc.tensor.matmul(
                                dst,
                                ones_sb[:, 0:128],
                                mi[:, off : off + CHUNK_COLS],
                                start=False,
                                stop=False,
                                skip_group_check=True,
                            )
                        )
                return P1, P2, thunks

            def rec_mm(P1t, P2t, P3t, i, vec, stop):
                """Accumulate Uk^T vec into step-i psum slices."""
                for j, m in ((0, M_R), (1, M_R + 1), (2, M_ZN), (3, M_ZN + 1)):
                    for k in range(KU):
                        nc.tensor.matmul(
                            P1t[:, j, i],
                            uk_sb[k][:, m * 128 : (m + 1) * 128],
                            vec[:, k],
                            start=False,
                            stop=(stop and k == KU - 1),
                            skip_group_check=True,
                        )
                for j in range(KU):
                    for k in range(KU):
                        nc.tensor.matmul(
                            P3t[:, j],
                            uk_sb[k][:, (M_XH + j) * 128 : (M_XH + j + 1) * 128],
                            vec[:, k],
                            start=False,
                            stop=False,
                            skip_group_check=True,
                        )

            def p3_start(P3t, vec):
                """First writers of a fresh P3 tile (ONE start=True total)."""
                for j in range(KU):
                    for k in range(KU):
                        nc.tensor.matmul(
                            P3t[:, j],
                            uk_sb[k][:, (M_XH + j) * 128 : (M_XH + j + 1) * 128],
                            vec[:, k],
                            start=(j == 0 and k == 0),
                            stop=False,
                            skip_group_check=True,
                        )

            def p3_const(P3t):
                for j in range(KU):
                    nc.tensor.matmul(
                        P3t[:, j],
                        hconst_sb[:, j * 128 : (j + 1) * 128],
                        ones_sb[:, :BLOC],
                        start=False,
                        stop=True,
                        skip_group_check=True,
                    )

            # ---- preamble: chunk 0 projection + step-0 u-part matmuls ----
            P1cur, P2cur, _th0 = proj(0)
            for _t in _th0:
                _t()
            P3next = p3p.tile([128, KU, BLOC], F32, tag="P3", name="P3")
            p3_start(P3next, u16v)
            # u-part into step-0 P1 (r+zn); v never comes for step 0 -> stop
            for j, m in ((0, M_R), (1, M_R + 1), (2, M_ZN), (3, M_ZN + 1)):
                for k in range(KU):
                    nc.tensor.matmul(
                        P1cur[:, j, 0],
                        uk_sb[k][:, m * 128 : (m + 1) * 128],
                        u16v[:, k],
                        start=False,
                        stop=(k == KU - 1),
                        skip_group_check=True,
                    )

            v16_prev = None

            for c in range(NCHUNKS):
                issue_dma((c * S) // DMA_STEPS + 2)
                P1nxt = P2nxt = None
                pthunks = []
                if c + 1 < NCHUNKS:
                    P1nxt, P2nxt, pthunks = proj(c + 1)
                for i in range(S):
                    g = c * S + i
                    P3 = P3next
                    # v-part of this step's recurrence (critical path mm)
                    if v16_prev is not None:
                        rec_mm(P1cur, P2cur, P3, i, v16_prev, stop=True)
                    p3_const(P3)

                    if DEBUG_DUMP:
                        for nm, src_ap, d1 in (("p1", P1cur[:, :, i], 4),
                                               ("p2", P2cur[:, :, i], KU),
                                               ("p3", P3[:, 0:2], KU)):
                            cp = gate.tile([128, d1, BLOC], F32, tag=f"dbgcp{nm}",
                                           name=f"dbgcp{nm}")
                            nc.vector.tensor_copy(out=cp, in_=src_ap)
                            nc.sync.dma_start(out=dbg[nm][g], in_=cp)
                    # copy mhh psum -> sbuf while ACT computes sigmoid(r):
                    # t16 then runs all-SBUF bf16 (2x DVE rate, cheap init)
                    mh16 = gate.tile([128, KU, BLOC], BF16, tag="mh16")
                    nc.vector.tensor_copy(out=mh16, in_=P3[:, 0:2])
                    # r first (waits only the r-block v-matmuls); q split
                    # off the critical path (needed only at v16)
                    r16 = gate.tile([128, KU, BLOC], BF16, tag="r16")
                    nc.scalar.activation(out=r16, in_=P1cur[:, 0:2, i], func=SIG)
                    q16 = gate.tile([128, KU, BLOC], BF16, tag="q16")
                    nc.scalar.activation(out=q16, in_=P1cur[:, 2:4, i], func=SIG)
                    t16 = gate.tile([128, KU, BLOC], BF16, tag="t16")
                    nc.vector.tensor_mul(out=t16, in0=r16, in1=mh16)
                    # hcp = xh + t16 via identity matmul accumulate (PE is
                    # cheaper than a DVE add on the critical path); one matmul
                    # per m-tile keeps each output AP 2D-contiguous
                    for j in range(KU):
                        nc.tensor.matmul(
                            P2cur[:, j, i],
                            identb,
                            t16[:, j],
                            start=False,
                            stop=True,
                            skip_group_check=True,
                        )
                    nc.scalar.activation(
                        out=btile[:, :, 1], in_=P2cur[:, :, i], func=SIG
                    )

                    # issue next step's u-part now: PE stays busy during the
                    # gate math and never blocks on it (u16 still = u(g-1))
                    if g + 1 < T:
                        P3next = p3p.tile([128, KU, BLOC], F32, tag="P3", name="P3")
                        p3_start(P3next, u16v)
                        if i + 1 < S:
                            P1t, it = P1cur, i + 1
                        else:
                            P1t, it = P1nxt, 0
                        for j, m in ((0, M_R), (1, M_R + 1), (2, M_ZN), (3, M_ZN + 1)):
                            for k in range(KU):
                                nc.tensor.matmul(
                                    P1t[:, j, it],
                                    uk_sb[k][:, m * 128 : (m + 1) * 128],
                                    u16v[:, k],
                                    start=False,
                                    stop=False,
                                    skip_group_check=True,
                                )


                    # drip-feed next chunk's projection into the PE
                    # idle window so it never blocks a rec_mm batch
                    n_th = (len(pthunks) + S - 1) // S if pthunks else 0
                    for _t in pthunks[i * n_th : (i + 1) * n_th]:
                        _t()

                    # 2q into the a-pattern (off critical path)
                    nc.vector.tensor_add(
                        out=atile[:, :, 2],
                        in0=q16.rearrange("p k b -> p (k b)"),
                        in1=q16.rearrange("p k b -> p (k b)"),
                    )
                    # v = q*(2s-u) in ONE scan op: state=u -> s-u/2 -> v
                    nc.vector.tensor_tensor_scan(
                        out=vtile.rearrange("p e t -> p (e t)"),
                        data0=atile.rearrange("p e t -> p (e t)"),
                        data1=btile.rearrange("p e t -> p (e t)"),
                        initial=0.0,
                        op0=mybir.AluOpType.mult,
                        op1=mybir.AluOpType.add,
                    )
                    v16 = vtile[:, :, 2].rearrange("p (k b) -> p k b", b=BLOC)
                    # state update (off critical path)
                    nc.vector.tensor_add(
                        out=u32,
                        in0=u32,
                        in1=v16,
                    )
                    nc.vector.tensor_copy(out=u16v, in_=u32)
                    v16_prev = v16
                    if DEBUG_DUMP:
                        for nm, t_ in (("t16", t16), ("v16", v16)):
                            nc.sync.dma_start(out=dbg[nm][g], in_=t_)
                P1cur, P2cur = P1nxt, P2nxt

            # ---- epilogue: h = u32 - 1, partition-major out (host transposes)
            hT = gate.tile([128, KU, BLOC], F32, tag="hT")
            nc.vector.tensor_scalar_sub(out=hT, in0=u32, scalar1=1.0)
            nc.sync.dma_start(out=out_d[:, :, :], in_=hT)

    nc.compile()
    return nc


_NC_CACHE = None


def _get_program():
    global _NC_CACHE
    if _NC_CACHE is None:
        _NC_CACHE = _build_program()
    return _NC_CACHE


def kernel(codes: np.ndarray, W: np.ndarray, Uk: np.ndarray, b: np.ndarray):
    codes = np.asarray(codes, dtype=np.float32)
    W = np.asarray(W, dtype=np.float32)
    Uk = np.asarray(Uk, dtype=np.float32)
    b = np.asarray(b, dtype=np.float32)

    Wz, Wr, Wh = W[:, :U], W[:, U : 2 * U], W[:, 2 * U :]
    Uz, Ur, Uh = Uk[:, :U], Uk[:, U : 2 * U], Uk[:, 2 * U :]
    b0, b1 = b[0], b[1]
    b0z, b0r, b0h = b0[:U], b0[U : 2 * U], b0[2 * U :]
    b1z, b1r, b1h = b1[:U], b1[U : 2 * U], b1[2 * U :]

    # device col-order: [xh(2*Wh), zn(-Wz), r(Wr)]
    w_dev = np.concatenate([2.0 * Wh, -Wz, Wr], axis=1)
    uk_dev = np.concatenate([2.0 * Uh, -Uz, Ur], axis=1)
    bias_dev = np.concatenate(
        [
            2.0 * b0h,
            -(b0z + b1z) + Uz.sum(axis=0),
            (b0r + b1r) - Ur.sum(axis=0),
        ]
    )[None, :]
    hconst = (2.0 * b1h - 2.0 * Uh.sum(axis=0))[None, :]

    w_in = np.ascontiguousarray(w_dev.reshape(KT, 128, 768).astype(ml_dtypes.bfloat16))
    uk_in = np.ascontiguousarray(uk_dev.reshape(KU, 128, 768).astype(ml_dtypes.bfloat16))
    bias_in = np.ascontiguousarray(bias_dev.astype(ml_dtypes.bfloat16))
    hconst_in = np.ascontiguousarray(hconst.astype(ml_dtypes.bfloat16))

    nc = _get_program()
    in_maps = []
    for cid in range(NCORES):
        sh = codes[cid * BLOC : (cid + 1) * BLOC]
        mask = np.any(sh != 0.0, axis=-1)
        minf = (-MASK_BIG * (1.0 - mask.astype(np.float32))).T.reshape(1, T * BLOC)
        codesT = (
            sh.transpose(2, 1, 0).reshape(KT, 128, T * BLOC).astype(ml_dtypes.bfloat16)
        )
        in_maps.append(
            {
                "codesT": np.ascontiguousarray(codesT),
                "minf": np.ascontiguousarray(minf.astype(ml_dtypes.bfloat16)),
                "w": w_in,
                "uk": uk_in,
                "bias": bias_in,
                "hconst": hconst_in,
            }
        )

    global LAST_RESULTS
    LAST_RESULTS = run_bass_kernel_spmd(
        nc,
        in_maps,
        list(range(NCORES)),
        trace=bool(int(os.environ.get("GRU_TRACE", "0"))),
    )
    outs = [
        r["out"].transpose(2, 1, 0).reshape(BLOC, U) for r in LAST_RESULTS.results
    ]
    return np.concatenate(outs, axis=0).astype(np.float32)


# revision 27
# speedup vs baseline: 1.0371x; 1.0371x over previous
"""Trainium2 Bass kernel for nn_GRULayer (Keras GRU, reset_after=True, Masking(0)).

Data-parallel over batch (8 cores, B_loc=32). Per core the 512-step scan is
latency-optimized; per step the critical chain is:

  v16 -> rec matmuls (v-part) -> sigmoid(r) -> t16 = r*mhh (DVE) ->
  ident-matmul accumulate (hcp = xh + t16, PE) -> sigmoid(s) ->
  tensor_tensor_scan (v = q*(2s-u), DVE) -> next step

Key tricks:
  - state u = h + 1 (f32 master u32; bf16 u at slot0 of the scan's b-tile);
    v(g) = u(g) - u(g-1). Recurrence matmul split: Uk^T u(g) =
    Uk^T u(g-1) [issued early, off path] + Uk^T v(g) [on path].
  - z-gate negated host-side: sigmoid gives q = 1-z; mask folds in as
    -30*(1-mask) so masked steps give q=0 (h carried).
  - candidate tanh via sigmoid: weights pre-scaled x2, hc = 2s-1; update
    u' = u + q*(2s - u) computed in ONE tensor_tensor_scan over
    [slot0, slot1, slot2] triples with a=[0,-0.5,2q], b=[u,s,0].
  - PSUM discipline: exactly one start=True per psum tile refill (the
    start arms the whole 2KB zero region; later matmuls init-then-
    accumulate per address).
  - projection, biases (ones-matmuls), mask, and next-step u-part matmuls
    all run in PE idle windows; projection is drip-fed between steps.

Host prep: z-block weights negated, candidate block scaled by 2, u-shift
constants (colsum of Uk) folded into biases. Output: h = u32 - 1,
PE-transposed to [batch, units].
"""

import os
import sys

sys.path.insert(0, "/opt/trn_rl_repo")

import ml_dtypes
import numpy as np

import concourse.bass as bass
import concourse.mybir as mybir
import concourse.tile as tile
from concourse import bacc
from concourse.bass_utils import run_bass_kernel_spmd
from concourse.masks import make_identity

B, T, D, U = 256, 512, 256, 256
NCORES = 8
BLOC = B // NCORES  # 32
KT = D // 128
KU = U // 128
S = 4  # steps per psum chunk
NCHUNKS = T // S  # 128
CHUNK_COLS = S * BLOC  # 128
DMA_STEPS = 32  # steps per DMA batch
DMA_COLS = DMA_STEPS * BLOC  # 1024
CHUNKS_PER_DMA = DMA_STEPS // S  # 8
N_DMA = T // DMA_STEPS  # 16
MASK_BIG = 30.0

F32 = mybir.dt.float32
BF16 = mybir.dt.bfloat16
SIG = mybir.ActivationFunctionType.Sigmoid

# column blocks in w/uk/bias (m-tile index):
#   0:2 -> candidate (2*Wh / 2*Ukh) -> P2[:,0:2]
#   2:4 -> zn (-Wz / -Ukz)          -> P2[:,2:4]
#   4:6 -> r  (Wr / Ukr)            -> P1
M_XH, M_ZN, M_R = 0, 2, 4

LAST_RESULTS = None  # test.py reads exec_time_ns off this
DEBUG_DUMP = False  # dump per-step gate intermediates to DRAM


def _build_program():
    nc = bacc.Bacc(
        "TRN2", target_bir_lowering=False, debug=False, num_devices=NCORES
    )

    codesT_d = nc.dram_tensor("codesT", [KT, 128, T * BLOC], BF16, kind="ExternalInput")
    minf_d = nc.dram_tensor("minf", [1, T * BLOC], BF16, kind="ExternalInput")
    w_d = nc.dram_tensor("w", [KT, 128, 768], BF16, kind="ExternalInput")
    uk_d = nc.dram_tensor("uk", [KU, 128, 768], BF16, kind="ExternalInput")
    bias_d = nc.dram_tensor("bias", [1, 768], BF16, kind="ExternalInput")
    hconst_d = nc.dram_tensor("hconst", [1, 256], BF16, kind="ExternalInput")
    out_d = nc.dram_tensor("out", [128, KU, BLOC], F32, kind="ExternalOutput")
    dbg = {}
    if DEBUG_DUMP:
        for nm, d1 in (("rq", 4), ("t16", KU), ("hcp16", KU), ("s16", KU),
                       ("w16", KU), ("v16", KU), ("u16d", KU)):
            dbg[nm] = nc.dram_tensor(f"dbg_{nm}", [T, 128, d1, BLOC], BF16,
                                     kind="ExternalOutput")
        for nm, d1 in (("p1", 4), ("p2", KU), ("p3", KU)):
            dbg[nm] = nc.dram_tensor(f"dbg_{nm}", [T, 128, d1, BLOC], F32,
                                     kind="ExternalOutput")

    with tile.TileContext(nc) as tc:
        with (
            tc.tile_pool(name="const", bufs=1) as const,
            tc.tile_pool(name="cin", bufs=3) as cin,
            tc.tile_pool(name="gate", bufs=3) as gate,
            tc.tile_pool(name="state", bufs=1) as state,
            tc.tile_pool(name="p1", bufs=2, space="PSUM") as p1p,
            tc.tile_pool(name="p2", bufs=2, space="PSUM") as p2p,
            tc.tile_pool(name="p3", bufs=2, space="PSUM") as p3p,
        ):
            # ---- constants ----
            w_sb = [const.tile([128, 768], BF16, tag=f"w{k}", name=f"w{k}") for k in range(KT)]
            uk_sb = [const.tile([128, 768], BF16, tag=f"uk{k}", name=f"uk{k}") for k in range(KU)]
            for k in range(KT):
                nc.sync.dma_start(out=w_sb[k], in_=w_d[k])
            for k in range(KU):
                nc.sync.dma_start(out=uk_sb[k], in_=uk_d[k])
            bias_sb = const.tile([1, 768], BF16, tag="bias")
            nc.sync.dma_start(out=bias_sb, in_=bias_d[0:1, :])
            hconst_sb = const.tile([1, 256], BF16, tag="hconst")
            nc.sync.dma_start(out=hconst_sb, in_=hconst_d[0:1, :])
            ones_sb = const.tile([1, CHUNK_COLS], BF16, tag="ones")
            nc.vector.memset(ones_sb, 1.0)
            identb = const.tile([128, 128], BF16, tag="identb")
            make_identity(nc, identb)

            # ---- state ----
            # tts layout: per element-triple [slot0, slot1, slot2]:
            #   a = [0, -0.5, 2q],  b = [u, s, 0]
            #   state: u -> s - u/2 = w/2 -> 2q*w/2 = q*(2s-u) = v
            u32 = state.tile([128, KU, BLOC], F32, tag="u32")
            nc.vector.memset(u32, 1.0)
            atile = state.tile([128, KU * BLOC, 3], BF16, tag="atile")
            nc.vector.memset(atile, 0.0)
            nc.vector.memset(atile[:, :, 1], -0.5)
            btile = state.tile([128, KU * BLOC, 3], BF16, tag="btile")
            nc.vector.memset(btile, 0.0)
            nc.vector.memset(btile[:, :, 0], 1.0)  # u0 = 1
            vtile = state.tile([128, KU * BLOC, 3], BF16, tag="vtile")
            # hc-scan staging: per element pair [slot0, slot1]:
            #   a2 = [0, r], xhs = [mhh, xh]  ->  state: mhh -> r*mhh + xh = hcp
            a2 = state.tile([128, KU, BLOC, 2], BF16, tag="a2")
            nc.vector.memset(a2, 0.0)
            hcs = state.tile([128, KU, BLOC, 2], BF16, tag="hcs")
            # dense-view helpers: u lives strided at slot 0
            u16v = btile[:, :, 0].rearrange("p (k b) -> p k b", b=BLOC)

            # ---- DMA prefetch machinery ----
            ct_bufs = {}

            def issue_dma(bidx):
                if bidx >= N_DMA or bidx in ct_bufs:
                    return
                d0 = bidx * DMA_COLS
                cts = [
                    cin.tile([128, DMA_COLS], BF16, tag=f"ct{k}", name=f"ct{k}")
                    for k in range(KT)
                ]
                for k in range(KT):
                    nc.sync.dma_start(out=cts[k], in_=codesT_d[k, :, d0 : d0 + DMA_COLS])
                mi = cin.tile([1, DMA_COLS], BF16, tag="mi")
                nc.sync.dma_start(out=mi, in_=minf_d[0:1, d0 : d0 + DMA_COLS])
                ct_bufs[bidx] = (cts, mi)

            issue_dma(0)
            issue_dma(1)

            def proj(c):
                """Allocate psum tiles for chunk c and write projection."""
                bidx = (c * S) // DMA_STEPS
                off = (c % CHUNKS_PER_DMA) * CHUNK_COLS
                cts, mi = ct_bufs[bidx]
                P1 = p1p.tile([128, 4, S, BLOC], F32, tag="P1")
                P2 = p2p.tile([128, KU, S, BLOC], F32, tag="P2")

                def pslice(j):
                    if j < 4:
                        return P1[:, j].rearrange("p s b -> p (s b)")
                    return P2[:, j - 4].rearrange("p s b -> p (s b)")

                # P1 j0:2 = r (m 4:6), P1 j2:4 = zn (m 2:4), P2 = xh (m 0:2)
                thunks = []
                for j, m in ((0, M_R), (1, M_R + 1), (2, M_ZN), (3, M_ZN + 1),
                             (4, M_XH), (5, M_XH + 1)):
                    dst = pslice(j)
                    for k in range(KT):
                        # exactly ONE start=True per psum tile refill: the
                        # first matmul arms the whole zero region; later
                        # writes init-then-accumulate per address
                        thunks.append(
                            lambda dst=dst, m=m, k=k, j=j: nc.tensor.matmul(
                                dst,
                                w_sb[k][:, m * 128 : (m + 1) * 128],
                                cts[k][:, off : off + CHUNK_COLS],
                                start=(j in (0, 4) and k == 0),
                                stop=False,
                                skip_group_check=True,
                            )
                        )
                    thunks.append(
                        lambda dst=dst, m=m: nc.tensor.matmul(
                            dst,
                            bias_sb[:, m * 128 : (m + 1) * 128],
                            ones_sb,
                            start=False,
                            stop=False,
                            skip_group_check=True,
                        )
                    )
                    if m in (M_ZN, M_ZN + 1):
                        thunks.append(
                            lambda dst=dst: nc.tensor.matmul(
                                dst,
                                ones_sb[:, 0:128],
                                mi[:, off : off + CHUNK_COLS],
                                start=False,
                                stop=False,
                                skip_group_check=True,
                            )
                        )
                return P1, P2, thunks

            def rec_mm(P1t, P2t, P3t, i, vec, stop):
                """Accumulate Uk^T vec into step-i psum slices."""
                for j, m in ((0, M_R), (1, M_R + 1), (2, M_ZN), (3, M_ZN + 1)):
                    for k in range(KU):
                        nc.tensor.matmul(
                            P1t[:, j, i],
                            uk_sb[k][:, m * 128 : (m + 1) * 128],
                            vec[:, k],
                            start=False,
                            stop=(stop and k == KU - 1),
                            skip_group_check=True,
                        )
                for j in range(KU):
                    for k in range(KU):
                        nc.tensor.matmul(
                            P3t[:, j],
                            uk_sb[k][:, (M_XH + j) * 128 : (M_XH + j + 1) * 128],
                            vec[:, k],
                            start=False,
                            stop=False,
                            skip_group_check=True,
                        )

            def p3_start(P3t, vec):
                """First writers of a fresh P3 tile (ONE start=True total)."""
                for j in range(KU):
                    for k in range(KU):
                        nc.tensor.matmul(
                            P3t[:, j],
                            uk_sb[k][:, (M_XH + j) * 128 : (M_XH + j + 1) * 128],
                            vec[:, k],
                            start=(j == 0 and k == 0),
                            stop=False,
                            skip_group_check=True,
                        )

            def p3_const(P3t):
                for j in range(KU):
                    nc.tensor.matmul(
                        P3t[:, j],
                        hconst_sb[:, j * 128 : (j + 1) * 128],
                        ones_sb[:, :BLOC],
                        start=False,
                        stop=True,
                        skip_group_check=True,
                    )

            # ---- preamble: chunk 0 projection + step-0 u-part matmuls ----
            P1cur, P2cur, _th0 = proj(0)
            for _t in _th0:
                _t()
            P3next = p3p.tile([128, KU, BLOC], F32, tag="P3", name="P3")
            p3_start(P3next, u16v)
            # u-part into step-0 P1 (r+zn); v never comes for step 0 -> stop
            for j, m in ((0, M_R), (1, M_R + 1), (2, M_ZN), (3, M_ZN + 1)):
                for k in range(KU):
                    nc.tensor.matmul(
                        P1cur[:, j, 0],
                        uk_sb[k][:, m * 128 : (m + 1) * 128],
                        u16v[:, k],
                        start=False,
                        stop=(k == KU - 1),
                        skip_group_check=True,
                    )

            v16_prev = None

            for c in range(NCHUNKS):
                issue_dma((c * S) // DMA_STEPS + 2)
                P1nxt = P2nxt = None
                pthunks = []
                if c + 1 < NCHUNKS:
                    P1nxt, P2nxt, pthunks = proj(c + 1)
                # stage this chunk's xh into SBUF slot1 (psum was projected
                # a chunk ahead; runs in the DVE idle window)
                xhs = gate.tile([128, S, KU, BLOC, 2], BF16, tag="xhs", bufs=2)
                nc.vector.tensor_copy(
                    out=xhs[:, :, :, :, 1],
                    in_=P2cur.rearrange("p j s b -> p s j b"),
                )
                for i in range(S):
                    g = c * S + i
                    P3 = P3next
                    # v-part of this step's recurrence (critical path mm)
                    if v16_prev is not None:
                        rec_mm(P1cur, P2cur, P3, i, v16_prev, stop=True)
                    p3_const(P3)

                    if DEBUG_DUMP:
                        for nm, src_ap, d1 in (("p1", P1cur[:, :, i], 4),
                                               ("p2", P2cur[:, :, i], KU),
                                               ("p3", P3[:, 0:2], KU)):
                            cp = gate.tile([128, d1, BLOC], F32, tag=f"dbgcp{nm}",
                                           name=f"dbgcp{nm}")
                            nc.vector.tensor_copy(out=cp, in_=src_ap)
                            nc.sync.dma_start(out=dbg[nm][g], in_=cp)
                    # copy mhh psum -> sbuf slot0 while ACT runs sigmoid(r)
                    nc.vector.tensor_copy(
                        out=xhs[:, i, :, :, 0], in_=P3[:, 0:2]
                    )
                    # r -> a2 slot1 (waits only the r-block v-matmuls); q
                    # split off the critical path (needed only at v16)
                    nc.scalar.activation(
                        out=a2[:, :, :, 1], in_=P1cur[:, 0:2, i], func=SIG
                    )
                    q16 = gate.tile([128, KU, BLOC], BF16, tag="q16")
                    nc.scalar.activation(out=q16, in_=P1cur[:, 2:4, i], func=SIG)
                    # hcp = r*mhh + xh in ONE scan: state: mhh -> r*mhh + xh
                    nc.vector.tensor_tensor_scan(
                        out=hcs.rearrange("p j b t -> p (j b t)"),
                        data0=a2.rearrange("p j b t -> p (j b t)"),
                        data1=xhs[:, i].rearrange("p j b t -> p (j b t)"),
                        initial=0.0,
                        op0=mybir.AluOpType.mult,
                        op1=mybir.AluOpType.add,
                    )
                    nc.scalar.activation(
                        out=btile[:, :, 1], in_=hcs[:, :, :, 1], func=SIG
                    )

                    # issue next step's u-part now: PE stays busy during the
                    # gate math and never blocks on it (u16 still = u(g-1))
                    if g + 1 < T:
                        P3next = p3p.tile([128, KU, BLOC], F32, tag="P3", name="P3")
                        p3_start(P3next, u16v)
                        if i + 1 < S:
                            P1t, it = P1cur, i + 1
                        else:
                            P1t, it = P1nxt, 0
                        for j, m in ((0, M_R), (1, M_R + 1), (2, M_ZN), (3, M_ZN + 1)):
                            for k in range(KU):
                                nc.tensor.matmul(
                                    P1t[:, j, it],
                                    uk_sb[k][:, m * 128 : (m + 1) * 128],
                                    u16v[:, k],
                                    start=False,
                                    stop=False,
                                    skip_group_check=True,
                                )


                    # drip-feed next chunk's projection into the PE
                    # idle window so it never blocks a rec_mm batch
                    n_th = (len(pthunks) + S - 1) // S if pthunks else 0
                    for _t in pthunks[i * n_th : (i + 1) * n_th]:
                        _t()

                    # 2q into the a-pattern (off critical path)
                    nc.vector.tensor_add(
                        out=atile[:, :, 2],
                        in0=q16.rearrange("p k b -> p (k b)"),
                        in1=q16.rearrange("p k b -> p (k b)"),
                    )
                    # v = q*(2s-u) in ONE scan op: state=u -> s-u/2 -> v
                    nc.vector.tensor_tensor_scan(
                        out=vtile.rearrange("p e t -> p (e t)"),
                        data0=atile.rearrange("p e t -> p (e t)"),
                        data1=btile.rearrange("p e t -> p (e t)"),
                        initial=0.0,
                        op0=mybir.AluOpType.mult,
                        op1=mybir.AluOpType.add,
                    )
                    v16 = vtile[:, :, 2].rearrange("p (k b) -> p k b", b=BLOC)
                    # state update (off critical path)
                    nc.vector.tensor_add(
                        out=u32,
                        in0=u32,
                        in1=v16,
                    )
                    nc.vector.tensor_copy(out=u16v, in_=u32)
                    v16_prev = v16
                    if DEBUG_DUMP:
                        nc.sync.dma_start(out=dbg["v16"][g], in_=v16)
                P1cur, P2cur = P1nxt, P2nxt

            # ---- epilogue: h = u32 - 1, partition-major out (host transposes)
            hT = gate.tile([128, KU, BLOC], F32, tag="hT")
            nc.vector.tensor_scalar_sub(out=hT, in0=u32, scalar1=1.0)
            nc.sync.dma_start(out=out_d[:, :, :], in_=hT)

    nc.compile()
    return nc


_NC_CACHE = None


def _get_program():
    global _NC_CACHE
    if _NC_CACHE is None:
        _NC_CACHE = _build_program()
    return _NC_CACHE


def kernel(codes: np.ndarray, W: np.ndarray, Uk: np.ndarray, b: np.ndarray):
    codes = np.asarray(codes, dtype=np.float32)
    W = np.asarray(W, dtype=np.float32)
    Uk = np.asarray(Uk, dtype=np.float32)
    b = np.asarray(b, dtype=np.float32)

    Wz, Wr, Wh = W[:, :U], W[:, U : 2 * U], W[:, 2 * U :]
    Uz, Ur, Uh = Uk[:, :U], Uk[:, U : 2 * U], Uk[:, 2 * U :]
    b0, b1 = b[0], b[1]
    b0z, b0r, b0h = b0[:U], b0[U : 2 * U], b0[2 * U :]
    b1z, b1r, b1h = b1[:U], b1[U : 2 * U], b1[2 * U :]

    # device col-order: [xh(2*Wh), zn(-Wz), r(Wr)]
    w_dev = np.concatenate([2.0 * Wh, -Wz, Wr], axis=1)
    uk_dev = np.concatenate([2.0 * Uh, -Uz, Ur], axis=1)
    bias_dev = np.concatenate(
        [
            2.0 * b0h,
            -(b0z + b1z) + Uz.sum(axis=0),
            (b0r + b1r) - Ur.sum(axis=0),
        ]
    )[None, :]
    hconst = (2.0 * b1h - 2.0 * Uh.sum(axis=0))[None, :]

    w_in = np.ascontiguousarray(w_dev.reshape(KT, 128, 768).astype(ml_dtypes.bfloat16))
    uk_in = np.ascontiguousarray(uk_dev.reshape(KU, 128, 768).astype(ml_dtypes.bfloat16))
    bias_in = np.ascontiguousarray(bias_dev.astype(ml_dtypes.bfloat16))
    hconst_in = np.ascontiguousarray(hconst.astype(ml_dtypes.bfloat16))

    nc = _get_program()
    in_maps = []
    for cid in range(NCORES):
        sh = codes[cid * BLOC : (cid + 1) * BLOC]
        mask = np.any(sh != 0.0, axis=-1)
        minf = (-MASK_BIG * (1.0 - mask.astype(np.float32))).T.reshape(1, T * BLOC)
        codesT = (
            sh.transpose(2, 1, 0).reshape(KT, 128, T * BLOC).astype(ml_dtypes.bfloat16)
        )
        in_maps.append(
            {
                "codesT": np.ascontiguousarray(codesT),
                "minf": np.ascontiguousarray(minf.astype(ml_dtypes.bfloat16)),
                "w": w_in,
                "uk": uk_in,
                "bias": bias_in,
                "hconst": hconst_in,
            }
        )

    global LAST_RESULTS
    LAST_RESULTS = run_bass_kernel_spmd(
        nc,
        in_maps,
        list(range(NCORES)),
        trace=bool(int(os.environ.get("GRU_TRACE", "0"))),
    )
    outs = [
        r["out"].transpose(2, 1, 0).reshape(BLOC, U) for r in LAST_RESULTS.results
    ]
    return np.concatenate(outs, axis=0).astype(np.float32)
